# revision 37
# baseline (speedup 1.0000x reference)
"""Multi-head scaled-dot-product attention (ABSA-style, per-head projections)
on 8 Trainium2 NeuronCores.

Reference computation (per head h, batch b):
    kx = k @ w_kx[h]                    # (512, 96)
    qx = q @ w_qx[h]                    # (512, 96)
    s  = qx @ kx.T / sqrt(96)           # (512, 512)
    a  = softmax(s, axis=-1)
    o  = a @ kx                         # (512, 96)
    out[b, :, h*96:(h+1)*96] = o

Distribution: data-parallel over batch. 32 batches are split 4-per-core over
8 cores; every core holds the full (tiny) weights and computes all 8 heads
for its 4 batches. No collectives needed — the host concatenates the
per-core outputs.

Measured (8 axon-tunneled TRN2 NeuronCores, trace runs): ~178-184 us NEFF
exec (vs ~204-235 us for the v1 kernel under the same harness), L2 rel
err 3.9e-3.  ~84% TensorE occupancy; the PE instruction stream itself
(~157 us) is the wall — projections/scores/attention all issue within a
few ns of their streaming floor.

Key structure:
  - q-projection stationaries zero-padded to 128 cols host-side (FWL).
  - k-projection flipped: stationary = kT seq-chunk [128,128] (full PE
    width), moving = 4-head weight block [128,384] -> kx lands NATURAL
    (heads on columns), feeding the attention matmul directly; per-head
    PE transposes then produce kx^T for the scores.
  - Scores for kc pairs land in one [128,2,512] PSUM tile (2 banks) so a
    single ACTIVATE computes exp() over 1024 columns — halves the per-
    instruction overhead on the scalar engine (the attention-phase gate).
  - Softmax denominator via a ones-column folded into the attention
    matmul (col 96 of the kx tile, 97-stride heads).
  - Normalisation: DVE evicts attn PSUM to SBUF, strided-gathers the 4
    denominators into one reciprocal, then does the per-qc scale-
    multiplies SBUF->SBUF (fast DVE mode, no PSUM read penalty).
  - Software pipelining: batch b's attention phase interleaves batch
    b+1's q-projection (qproj(b+1,h) -> scores(h+1) -> attn(h) per
    slot, scores prewarmed one head at the end of phase K); batch
    b+1's input transposes interleave batch b's k-projection chains.
  - PSUM: scores 2x2 banks, shared pool (kproj/qproj/attn) 2 banks,
    transposes 2 banks = 8.
  - All PSUM evictions on the vector engine; scalar does exp() only.
"""

import math
from functools import lru_cache

import numpy as np

import concourse.bass as bass
import concourse.tile as tile
from concourse import mybir
from concourse.bass_utils import run_bass_kernel_spmd
from concourse.masks import make_identity

# ---------------------------------------------------------------------------
# Workaround for walrus "Too many sync wait commands": some instruction
# encodings accept only a single sync-wait, but Tile can attach several.
# Hoist every wait beyond the first onto a same-engine no-op inserted right
# before the instruction — program order on the engine makes that equivalent.
# ---------------------------------------------------------------------------

import bass_rust as _bass_rust


def _split_excess_waits(nc, max_waits=1):
    n = 0
    for f in nc.m.functions:
        for bb in f.blocks:
            il = bb.instructions
            i = 0
            while i < len(il):
                ins = il[i]
                si = ins.sync_info
                waits = list(si.on_wait or []) if si is not None else []
                if len(waits) > max_waits:
                    si.on_wait = waits[:max_waits]
                    for w in waits[max_waits:]:
                        nop = mybir.InstNoOp(name=f"waitnop-{n}", ins=[],
                                             outs=[])
                        n += 1
                        nop.engine = ins.engine
                        nop.sync_info = _bass_rust.SyncInfo(
                            on_wait=[w], on_update=[])
                        il.insert(i, nop)
                        i += 1
                i += 1

# ---------------------------------------------------------------------------
# Problem constants (full problem; hardcoded per the harness contract)
# ---------------------------------------------------------------------------
EMBED = 768
HID = 96
N_HEAD = 8
BATCH = 32
SEQ = 512
N_CORES = 8
B = BATCH // N_CORES  # batches per core
EC = EMBED // 128  # embed chunks of 128
KC = SEQ // 128  # key (seq) chunks of 128
QC = SEQ // 128  # query chunks of 128
SCALE = 1.0 / math.sqrt(HID)
HP = HID + 1  # per-head kxo stride: 96 data cols + 1 ones col

F32 = mybir.dt.float32
BF16 = mybir.dt.bfloat16


def build_bass():
    nc = bass.Bass("TRN2", target_bir_lowering=False, debug=False,
                   num_devices=N_CORES)

    k_in = nc.declare_dram_parameter("k", [B, SEQ, EMBED], F32, isOutput=False)
    q_in = nc.declare_dram_parameter("q", [B, SEQ, EMBED], F32, isOutput=False)
    # host-packed weights:
    #   w_kx: [128, EC, N_HEAD*HID]  (p, ec, h*96+d) = w_kx[h, ec*128+p, d]
    #   w_qx: [128, N_HEAD*EC, 128]  (p, h*6+ec, d)  = w_qx[h, ec*128+p, d],
    #         d-padded 96->128 with zeros (FWL wants 128 weight columns)
    wk_in = nc.declare_dram_parameter("w_kx", [128, EC, N_HEAD * HID], F32,
                                      isOutput=False)
    wq_in = nc.declare_dram_parameter("w_qx", [128, N_HEAD * EC, 128], F32,
                                      isOutput=False)
    out_d = nc.declare_dram_parameter("out", [B, SEQ, EMBED], F32,
                                      isOutput=True)

    with nc.allow_low_precision("bf16 compute, f32 accumulate"), \
            tile.TileContext(nc) as tc:
        with tc.tile_pool(name="singles", bufs=1) as singles, \
                tc.tile_pool(name="nat", bufs=4) as nat_pool, \
                tc.tile_pool(name="kqt", bufs=1) as kqt_pool, \
                tc.tile_pool(name="wsb", bufs=1) as w_pool, \
                tc.tile_pool(name="stage", bufs=1) as stage_pool, \
                tc.tile_pool(name="exp", bufs=6) as exp_pool, \
                tc.tile_pool(name="osb", bufs=4) as osb_pool, \
                tc.tile_pool(name="recip", bufs=8) as recip_pool, \
                tc.tile_pool(name="ps_s", bufs=2, space="PSUM") as ps_s, \
                tc.tile_pool(name="ps_m", bufs=2, space="PSUM") as ps_m, \
                tc.tile_pool(name="ps_tr", bufs=2, space="PSUM") as ps_tr:

            # --- SBUF tiles -----------------------------------------------
            wq_sb = w_pool.tile([128, N_HEAD * EC, 128], BF16, tag="wq",
                                name="wq_sb")
            wk_sb = w_pool.tile([128, EC, N_HEAD * HID], BF16, tag="wk",
                                name="wk_sb")
            identity = singles.tile([128, 128], BF16, tag="identity")

            # qxT / kxT: head-PAIR tiles [96, 2, 512] bf16 (hid on parts).
            qxT_p = [[singles.tile([HID, 2, SEQ], BF16, tag=f"qxT_{i}_{hp}",
                                   name=f"qxT_{i}_{hp}")
                      for hp in range(N_HEAD // 2)] for i in range(2)]
            # kxo: per (parity, seq-chunk) [128, N_HEAD, 97] bf16 — kx in
            # natural layout, ones column at 96 (softmax denominator).
            kxo = [[singles.tile([128, N_HEAD, HP], BF16,
                                 tag=f"kxo_{i}_{sc}", name=f"kxo_{i}_{sc}")
                    for sc in range(KC)] for i in range(2)]
            kxT_p = [[singles.tile([HID, 2, SEQ], BF16, tag=f"kxT_{i}_{hp}",
                                   name=f"kxT_{i}_{hp}")
                      for hp in range(N_HEAD // 2)] for i in range(2)]
            stage = [[stage_pool.tile([128, EMBED], F32, tag=f"st{p}_{qc}",
                                      name=f"st{p}_{qc}")
                      for qc in range(QC)] for p in range(2)]

            def qxT(b, h):
                return qxT_p[b % 2][h // 2][:, h % 2, :]

            def kxT(b, h):
                return kxT_p[b % 2][h // 2][:, h % 2, :]

            # --- input pipeline -------------------------------------------
            # SWDGE cast-DMAs (f32 -> bf16, contiguous descriptors).  Queue
            # order: wq half 0 first (smallest gating load), then q0, then
            # identity prep, k0, wq half 1, wk, batches 1..3.
            def load_wq(half):
                hb = N_HEAD * EC // 2
                sl = slice(half * hb, (half + 1) * hb)
                nc.gpsimd.dma_start(out=wq_sb[:, sl, :], in_=wq_in[:, sl, :])

            def load_wk():
                nc.gpsimd.dma_start(out=wk_sb[:], in_=wk_in[:])

            def cast_batch_tensor(b, t, split=False):
                src_d = (k_in, q_in)[t]
                nat = nat_pool.tile([128, KC, EMBED], BF16,
                                    tag=f"nat{t}", name=f"nat{t}_{b}")
                src = src_d[b].rearrange("(kc p) e -> p kc e", p=128)
                if split:
                    # two cast-DMAs so the first seq-half lands ~2us
                    # earlier and its transposes can start (Tile's
                    # region-level deps release them per kc chunk)
                    half = KC // 2
                    nc.gpsimd.dma_start(out=nat[:, 0:half, :],
                                        in_=src[:, 0:half, :])
                    nc.gpsimd.dma_start(out=nat[:, half:, :],
                                        in_=src[:, half:, :])
                else:
                    nc.gpsimd.dma_start(out=nat[:], in_=src[:])
                return nat

            load_wq(0)
            nat_q0 = cast_batch_tensor(0, 1, split=True)
            make_identity(nc, identity[:])
            load_wq(1)
            nat_k0 = cast_batch_tensor(0, 0, split=True)
            load_wk()
            nats = {(0, 1): nat_q0, (0, 0): nat_k0}
            for b in range(1, B):
                for t in (1, 0):
                    nats[(b, t)] = cast_batch_tensor(b, t)

            # PE warm-up transposes: keep the PE busy from engine-ready to
            # first real matmul so the HAM clock gate flips to 2.4 GHz and
            # stays there.
            warm_ps = ps_s.tile([128, 256], BF16, tag="s", name="warm_ps")
            for _ in range(88):
                nc.tensor.transpose(warm_ps[:, 0:128], identity[:],
                                    identity[:])

            # kT/qT (embed on partitions) built with PE transposes, stored
            # as ec-PAIR tiles [128, 2, 512] bf16.
            kqt = {}

            def inT(b, t, ec):
                return kqt[(b, t, ec // 2)][:, ec % 2, :]

            def input_transpose_pair(b, t, nat, ep, evict_on_scalar=False):
                tp = ps_tr.tile([128, 2, KC, 128], BF16, tag="tr",
                                name="in_tr")
                for e2 in range(2):
                    for kc in range(KC):
                        nc.tensor.transpose(
                            tp[:, e2, kc, :],
                            nat[:, kc, (ep * 2 + e2) * 128:
                                (ep * 2 + e2 + 1) * 128],
                            identity[:])
                tt = kqt_pool.tile([128, 2, SEQ], BF16,
                                   tag=f"T{t}_{b}_{ep}",
                                   name=f"T{t}_{b}_{ep}")
                if evict_on_scalar:
                    nc.scalar.copy(tt[:], tp[:])
                else:
                    nc.vector.tensor_copy(tt[:], tp[:])
                kqt[(b, t, ep)] = tt

            # --- phase building blocks ------------------------------------
            def qproj_head(b, h):
                # qx^T[h] via padded stationary wq chunk [128,128]:
                # psum rows 0:96 = qx^T, rows 96:128 = zeros (pad).
                qp = ps_m.tile([128, SEQ], F32, tag="m", name="qproj_ps")
                for ec in range(EC):
                    nc.tensor.matmul(qp[:], wq_sb[:, h * EC + ec, :],
                                     inT(b, 1, ec),
                                     start=(ec == 0), stop=(ec == EC - 1))
                nc.vector.tensor_copy(qxT(b, h), qp[0:HID, :])

            def kproj_chain(b, sc, half):
                # flipped projection: stationary kT seq-chunk [128,128],
                # moving = 4-head weight block [128, 384]; lands kx natural.
                dst = kxo[b % 2][sc]
                hs = slice(half * 4 * HID, (half + 1) * 4 * HID)
                kp = ps_m.tile([128, 4, HID], F32, tag="m", name="kproj_ps")
                for ec in range(EC):
                    nc.tensor.matmul(
                        kp[:],
                        inT(b, 0, ec)[:, sc * 128:(sc + 1) * 128],
                        wk_sb[:, ec, hs],
                        start=(ec == 0), stop=(ec == EC - 1))
                nc.vector.tensor_copy(
                    dst[:, half * 4:(half + 1) * 4, 0:HID], kp[:])
                if half == 1:
                    nc.gpsimd.memset(dst[:, :, HID:HP], 1.0)

            def kxT_transpose_pair(b, hp):
                par = b % 2
                tp = ps_tr.tile([HID, 2, SEQ], BF16, tag="tr", name="kxT_tr")
                for h2 in range(2):
                    for sc in range(KC):
                        nc.tensor.transpose(
                            tp[:, h2, sc * 128:(sc + 1) * 128],
                            kxo[par][sc][:, hp * 2 + h2, 0:HID],
                            identity[:])
                nc.vector.tensor_copy(kxT_p[par][hp][:], tp[:])

            def scores_exp(b, h):
                # s^T (k on psum partitions) in kc-pair tiles; one exp()
                # per pair over 1024 columns.
                exp_sb = []
                for pair in range(2):
                    s2 = ps_s.tile([128, 2, SEQ], F32, tag="s",
                                   name="score_ps")
                    for k2 in range(2):
                        sc = pair * 2 + k2
                        nc.tensor.matmul(
                            s2[:, k2, :],
                            kxT(b, h)[:, sc * 128:(sc + 1) * 128],
                            qxT(b, h), start=True, stop=True)
                    e2 = exp_pool.tile([128, 2, SEQ], BF16, tag="exp",
                                       name="e_sb")
                    nc.scalar.activation(
                        e2[:], s2[:],
                        mybir.ActivationFunctionType.Exp, scale=SCALE)
                    exp_sb.extend([e2[:, 0, :], e2[:, 1, :]])
                return exp_sb

            def attn_head(b, h, exp_sb, store_final=False):
                # attention-weighted values + softmax denominator (col 96),
                # one [128, 4, 97] psum tile per head (all 4 q chunks);
                # normalisation: DVE evicts, gathers the 4 reciprocals in
                # one op, then scale-multiplies into the staging tile.
                # store_final: issue each qc's second-half store as soon as
                # its normalisation lands (trims the drain tail).
                par = b % 2
                o_ps = ps_m.tile([128, QC, HP], F32, tag="m", name="o_ps")
                for qc in range(QC):
                    for kc in range(KC):
                        nc.tensor.matmul(
                            o_ps[:, qc, :],
                            exp_sb[kc][:, qc * 128:(qc + 1) * 128],
                            kxo[par][kc][:, h, :],
                            start=(kc == 0), stop=(kc == KC - 1))
                if store_final:
                    # latency-optimised drain: per-qc recip/scale straight
                    # from PSUM (scale alternating DVE/ACT) + immediate
                    # store, so the final stores overlap the remaining
                    # normalisations
                    for qc in range(QC):
                        dst = stage[par][qc][:, h * HID:(h + 1) * HID]
                        rc = recip_pool.tile([128, 1], F32, tag="recip",
                                             name="recip")
                        nc.vector.reciprocal(rc[:], o_ps[:, qc, HID:HP])
                        if qc % 2:
                            nc.scalar.mul(dst, o_ps[:, qc, 0:HID], rc[:])
                        else:
                            nc.vector.tensor_scalar_mul(
                                dst, o_ps[:, qc, 0:HID], rc[:])
                        # alternate store queues (sync/scalar HWDGE) so the
                        # four ~600ns DMA issues don't serialize on one
                        # engine during the drain
                        eng = nc.scalar if qc % 2 else nc.sync
                        eng.dma_start(
                            out=out_d[b, qc * 128:(qc + 1) * 128,
                                      6 * HID:],
                            in_=stage[par][qc][:, 6 * HID:])
                    return
                o_sb = osb_pool.tile([128, QC, HP], F32, tag="osb",
                                     name="o_sb")
                nc.vector.tensor_copy(o_sb[:], o_ps[:])
                rc = recip_pool.tile([128, QC], F32, tag="recip",
                                     name="recip")
                nc.vector.reciprocal(rc[:], o_sb[:, :, HID:HP])
                for qc in range(QC):
                    nc.vector.tensor_scalar_mul(
                        stage[par][qc][:, h * HID:(h + 1) * HID],
                        o_sb[:, qc, 0:HID], rc[:, qc:qc + 1])

            def store_cols(b, lo, hi):
                par = b % 2
                sl = slice(lo, hi)
                for qc in range(QC):
                    nc.sync.dma_start(
                        out=out_d[b, qc * 128:(qc + 1) * 128, sl],
                        in_=stage[par][qc][:, sl])

            # --- emission -------------------------------------------------
            # batch 0 prologue (nothing earlier to hide behind); k-side
            # input transposes interleave the last q-projection heads so
            # their evictions overlap PE work.
            for ep in range(EC // 2):
                # alternate eviction engines so the three qT evictions
                # finish ~in parallel and the first q-projection can start
                input_transpose_pair(0, 1, nats[(0, 1)], ep,
                                     evict_on_scalar=(ep == 1))
            for h in range(N_HEAD):
                qproj_head(0, h)
                if h in (4, 5, 6):
                    # after the k cast has landed; evictions on the
                    # (startup-idle) scalar engine so the DVE's qxT
                    # eviction backlog can't delay kproj
                    input_transpose_pair(0, 0, nats[(0, 0)], h - 4,
                                         evict_on_scalar=True)

            for b in range(B):
                # PHASE K(b): k-projection chains interleaved with batch
                # b+1's input transposes (hides DVE eviction latency), then
                # per-head-pair kx^T transposes.
                it = [(b + 1, t, nats[(b + 1, t)], ep)
                      for t in (1, 0) for ep in range(EC // 2)] \
                    if b + 1 < B else []
                ic = 0
                for sc in range(KC):
                    for half in range(2):
                        kproj_chain(b, sc, half)
                        if (sc * 2 + half) % 3 and ic < len(it):
                            input_transpose_pair(*it[ic])
                            ic += 1
                while ic < len(it):
                    input_transpose_pair(*it[ic])
                    ic += 1
                kxT_transpose_pair(b, 0)
                kxT_transpose_pair(b, 1)
                # prewarm: head 0's scores at the end of phase K give the
                # scalar engine (idle during K) a one-head exp head-start
                exps = {0: scores_exp(b, 0)}
                for hp in range(2, N_HEAD // 2):
                    kxT_transpose_pair(b, hp)

                # PHASE A(b): qproj(b+1,h) -> scores(h+1) -> attn(h) per
                # slot; scores stay one head ahead of attn so exp latency
                # is covered by the interleaved projection work.
                for h in range(N_HEAD):
                    if b + 1 < B:
                        qproj_head(b + 1, h)
                    if h + 1 < N_HEAD:
                        exps[h + 1] = scores_exp(b, h + 1)
                    attn_head(b, h, exps.pop(h),
                              store_final=(h == N_HEAD - 1))
                    if h == 3:
                        store_cols(b, 0, EMBED // 2)
                    elif h == 5:
                        store_cols(b, EMBED // 2, 6 * HID)

    _split_excess_waits(nc)
    return nc


@lru_cache(maxsize=1)
def _get_nc():
    return build_bass()


def _pack_weights(w_kx, w_qx):
    # w_kx [8, 768, 96] -> [128, 6, 768]: (p, ec, h*96+d)
    wk = np.ascontiguousarray(
        w_kx.reshape(N_HEAD, EC, 128, HID).transpose(2, 1, 0, 3)
        .reshape(128, EC, N_HEAD * HID), dtype=np.float32)
    # w_qx [8, 768, 96] -> [128, 48, 128] zero-padded: (p, h*6+ec, d)
    wq = np.zeros((128, N_HEAD, EC, 128), dtype=np.float32)
    wq[:, :, :, 0:HID] = w_qx.reshape(N_HEAD, EC, 128, HID).transpose(
        2, 0, 1, 3)
    wq = np.ascontiguousarray(wq.reshape(128, N_HEAD * EC, 128))
    return wk, wq


def kernel(k, q, w_kx, w_qx):
    k = np.ascontiguousarray(k, dtype=np.float32)
    q = np.ascontiguousarray(q, dtype=np.float32)
    w_kx = np.ascontiguousarray(w_kx, dtype=np.float32)
    w_qx = np.ascontiguousarray(w_qx, dtype=np.float32)
    wk_packed, wq_packed = _pack_weights(w_kx, w_qx)

    nc = _get_nc()
    in_maps = []
    for c in range(N_CORES):
        sl = slice(c * B, (c + 1) * B)
        in_maps.append({
            "k": np.ascontiguousarray(k[sl]),
            "q": np.ascontiguousarray(q[sl]),
            "w_kx": wk_packed,
            "w_qx": wq_packed,
        })
    res = run_bass_kernel_spmd(nc, in_maps, core_ids=list(range(N_CORES)))
    return np.concatenate([res.results[c]["out"] for c in range(N_CORES)],
                          axis=0)


# revision 39
# speedup vs baseline: 1.0159x; 1.0159x over previous
"""Multi-head scaled-dot-product attention (ABSA-style, per-head projections)
on 8 Trainium2 NeuronCores.

Reference computation (per head h, batch b):
    kx = k @ w_kx[h]                    # (512, 96)
    qx = q @ w_qx[h]                    # (512, 96)
    s  = qx @ kx.T / sqrt(96)           # (512, 512)
    a  = softmax(s, axis=-1)
    o  = a @ kx                         # (512, 96)
    out[b, :, h*96:(h+1)*96] = o

Distribution: data-parallel over batch. 32 batches are split 4-per-core over
8 cores; every core holds the full (tiny) weights and computes all 8 heads
for its 4 batches. No collectives needed — the host concatenates the
per-core outputs.

Measured (8 axon-tunneled TRN2 NeuronCores, trace runs): ~178-184 us NEFF
exec (vs ~204-235 us for the v1 kernel under the same harness), L2 rel
err 3.9e-3.  ~84% TensorE occupancy; the PE instruction stream itself
(~157 us) is the wall — projections/scores/attention all issue within a
few ns of their streaming floor.

Key structure:
  - q-projection stationaries zero-padded to 128 cols host-side (FWL).
  - k-projection flipped: stationary = kT seq-chunk [128,128] (full PE
    width), moving = 4-head weight block [128,384] -> kx lands NATURAL
    (heads on columns), feeding the attention matmul directly; per-head
    PE transposes then produce kx^T for the scores.
  - Scores for kc pairs land in one [128,2,512] PSUM tile (2 banks) so a
    single ACTIVATE computes exp() over 1024 columns — halves the per-
    instruction overhead on the scalar engine (the attention-phase gate).
  - Softmax denominator via a ones-column folded into the attention
    matmul (col 96 of the kx tile, 97-stride heads).
  - Normalisation: DVE evicts attn PSUM to SBUF, strided-gathers the 4
    denominators into one reciprocal, then does the per-qc scale-
    multiplies SBUF->SBUF (fast DVE mode, no PSUM read penalty).
  - Software pipelining: batch b's attention phase interleaves batch
    b+1's q-projection (qproj(b+1,h) -> scores(h+1) -> attn(h) per
    slot, scores prewarmed one head at the end of phase K); batch
    b+1's input transposes interleave batch b's k-projection chains.
  - PSUM: scores 2x2 banks, shared pool (kproj/qproj/attn) 2 banks,
    transposes 2 banks = 8.
  - All PSUM evictions on the vector engine; scalar does exp() only.
"""

import math
from functools import lru_cache

import numpy as np

import concourse.bass as bass
import concourse.tile as tile
from concourse import mybir
from concourse.bass_utils import run_bass_kernel_spmd
from concourse.masks import make_identity

# ---------------------------------------------------------------------------
# Workaround for walrus "Too many sync wait commands": some instruction
# encodings accept only a single sync-wait, but Tile can attach several.
# Hoist every wait beyond the first onto a same-engine no-op inserted right
# before the instruction — program order on the engine makes that equivalent.
# ---------------------------------------------------------------------------

import bass_rust as _bass_rust


def _split_excess_waits(nc, max_waits=1):
    n = 0
    for f in nc.m.functions:
        for bb in f.blocks:
            il = bb.instructions
            i = 0
            while i < len(il):
                ins = il[i]
                si = ins.sync_info
                waits = list(si.on_wait or []) if si is not None else []
                if len(waits) > max_waits:
                    si.on_wait = waits[:max_waits]
                    for w in waits[max_waits:]:
                        nop = mybir.InstNoOp(name=f"waitnop-{n}", ins=[],
                                             outs=[])
                        n += 1
                        nop.engine = ins.engine
                        nop.sync_info = _bass_rust.SyncInfo(
                            on_wait=[w], on_update=[])
                        il.insert(i, nop)
                        i += 1
                i += 1

# ---------------------------------------------------------------------------
# Problem constants (full problem; hardcoded per the harness contract)
# ---------------------------------------------------------------------------
EMBED = 768
HID = 96
N_HEAD = 8
BATCH = 32
SEQ = 512
N_CORES = 8
B = BATCH // N_CORES  # batches per core
EC = EMBED // 128  # embed chunks of 128
KC = SEQ // 128  # key (seq) chunks of 128
QC = SEQ // 128  # query chunks of 128
SCALE = 1.0 / math.sqrt(HID)
HP = HID + 1  # per-head kxo stride: 96 data cols + 1 ones col

F32 = mybir.dt.float32
BF16 = mybir.dt.bfloat16


def build_bass():
    nc = bass.Bass("TRN2", target_bir_lowering=False, debug=False,
                   num_devices=N_CORES)

    k_in = nc.declare_dram_parameter("k", [B, SEQ, EMBED], F32, isOutput=False)
    q_in = nc.declare_dram_parameter("q", [B, SEQ, EMBED], F32, isOutput=False)
    # host-packed weights:
    #   w_kx: [128, EC, N_HEAD*HID]  (p, ec, h*96+d) = w_kx[h, ec*128+p, d]
    #   w_qx: [128, N_HEAD*EC, 128]  (p, h*6+ec, d)  = w_qx[h, ec*128+p, d],
    #         d-padded 96->128 with zeros (FWL wants 128 weight columns)
    wk_in = nc.declare_dram_parameter("w_kx", [128, EC, N_HEAD * HID], F32,
                                      isOutput=False)
    wq_in = nc.declare_dram_parameter("w_qx", [128, N_HEAD * EC, 128], F32,
                                      isOutput=False)
    out_d = nc.declare_dram_parameter("out", [B, SEQ, EMBED], F32,
                                      isOutput=True)

    with nc.allow_low_precision("bf16 compute, f32 accumulate"), \
            tile.TileContext(nc) as tc:
        with tc.tile_pool(name="singles", bufs=1) as singles, \
                tc.tile_pool(name="nat", bufs=4) as nat_pool, \
                tc.tile_pool(name="kqt", bufs=1) as kqt_pool, \
                tc.tile_pool(name="wsb", bufs=1) as w_pool, \
                tc.tile_pool(name="stage", bufs=1) as stage_pool, \
                tc.tile_pool(name="exp", bufs=6) as exp_pool, \
                tc.tile_pool(name="osb", bufs=4) as osb_pool, \
                tc.tile_pool(name="recip", bufs=8) as recip_pool, \
                tc.tile_pool(name="ps_s", bufs=2, space="PSUM") as ps_s, \
                tc.tile_pool(name="ps_m", bufs=2, space="PSUM") as ps_m, \
                tc.tile_pool(name="ps_tr", bufs=2, space="PSUM") as ps_tr:

            # --- SBUF tiles -----------------------------------------------
            wq_sb = w_pool.tile([128, N_HEAD * EC, 128], BF16, tag="wq",
                                name="wq_sb")
            wk_sb = w_pool.tile([128, EC, N_HEAD * HID], BF16, tag="wk",
                                name="wk_sb")
            identity = singles.tile([128, 128], BF16, tag="identity")

            # qxT / kxT: head-PAIR tiles [96, 2, 512] bf16 (hid on parts).
            qxT_p = [[singles.tile([HID, 2, SEQ], BF16, tag=f"qxT_{i}_{hp}",
                                   name=f"qxT_{i}_{hp}")
                      for hp in range(N_HEAD // 2)] for i in range(2)]
            # kxo: per (parity, seq-chunk) [128, N_HEAD, 97] bf16 — kx in
            # natural layout, ones column at 96 (softmax denominator).
            kxo = [[singles.tile([128, N_HEAD, HP], BF16,
                                 tag=f"kxo_{i}_{sc}", name=f"kxo_{i}_{sc}")
                    for sc in range(KC)] for i in range(2)]
            kxT_p = [[singles.tile([HID, 2, SEQ], BF16, tag=f"kxT_{i}_{hp}",
                                   name=f"kxT_{i}_{hp}")
                      for hp in range(N_HEAD // 2)] for i in range(2)]
            stage = [[stage_pool.tile([128, EMBED], F32, tag=f"st{p}_{qc}",
                                      name=f"st{p}_{qc}")
                      for qc in range(QC)] for p in range(2)]

            def qxT(b, h):
                return qxT_p[b % 2][h // 2][:, h % 2, :]

            def kxT(b, h):
                return kxT_p[b % 2][h // 2][:, h % 2, :]

            # --- input pipeline -------------------------------------------
            # SWDGE cast-DMAs (f32 -> bf16, contiguous descriptors).  Queue
            # order: wq half 0 first (smallest gating load), then q0, then
            # identity prep, k0, wq half 1, wk, batches 1..3.
            def load_wq(half):
                hb = N_HEAD * EC // 2
                sl = slice(half * hb, (half + 1) * hb)
                nc.gpsimd.dma_start(out=wq_sb[:, sl, :], in_=wq_in[:, sl, :])

            def load_wk():
                nc.gpsimd.dma_start(out=wk_sb[:], in_=wk_in[:])

            def cast_batch_tensor(b, t, split=False):
                src_d = (k_in, q_in)[t]
                nat = nat_pool.tile([128, KC, EMBED], BF16,
                                    tag=f"nat{t}", name=f"nat{t}_{b}")
                src = src_d[b].rearrange("(kc p) e -> p kc e", p=128)
                if split:
                    # two cast-DMAs so the first seq-half lands ~2us
                    # earlier and its transposes can start (Tile's
                    # region-level deps release them per kc chunk)
                    half = KC // 2
                    nc.gpsimd.dma_start(out=nat[:, 0:half, :],
                                        in_=src[:, 0:half, :])
                    nc.gpsimd.dma_start(out=nat[:, half:, :],
                                        in_=src[:, half:, :])
                else:
                    nc.gpsimd.dma_start(out=nat[:], in_=src[:])
                return nat

            load_wq(0)
            nat_q0 = cast_batch_tensor(0, 1)
            make_identity(nc, identity[:])
            load_wq(1)
            nat_k0 = cast_batch_tensor(0, 0)
            load_wk()
            nats = {(0, 1): nat_q0, (0, 0): nat_k0}
            for b in range(1, B):
                for t in (1, 0):
                    nats[(b, t)] = cast_batch_tensor(b, t)

            # PE warm-up transposes: keep the PE busy from engine-ready to
            # first real matmul so the HAM clock gate flips to 2.4 GHz and
            # stays there.
            warm_ps = ps_s.tile([128, 256], BF16, tag="s", name="warm_ps")
            for _ in range(88):
                nc.tensor.transpose(warm_ps[:, 0:128], identity[:],
                                    identity[:])

            # kT/qT (embed on partitions) built with PE transposes, stored
            # as ec-PAIR tiles [128, 2, 512] bf16.
            kqt = {}

            def inT(b, t, ec):
                return kqt[(b, t, ec // 2)][:, ec % 2, :]

            def input_transpose_pair(b, t, nat, ep, evict_on_scalar=False):
                tp = ps_tr.tile([128, 2, KC, 128], BF16, tag="tr",
                                name="in_tr")
                for e2 in range(2):
                    for kc in range(KC):
                        nc.tensor.transpose(
                            tp[:, e2, kc, :],
                            nat[:, kc, (ep * 2 + e2) * 128:
                                (ep * 2 + e2 + 1) * 128],
                            identity[:])
                tt = kqt_pool.tile([128, 2, SEQ], BF16,
                                   tag=f"T{t}_{b}_{ep}",
                                   name=f"T{t}_{b}_{ep}")
                if evict_on_scalar:
                    nc.scalar.copy(tt[:], tp[:])
                else:
                    nc.vector.tensor_copy(tt[:], tp[:])
                kqt[(b, t, ep)] = tt

            # --- phase building blocks ------------------------------------
            def qproj_head(b, h):
                # qx^T[h] via padded stationary wq chunk [128,128]:
                # psum rows 0:96 = qx^T, rows 96:128 = zeros (pad).
                qp = ps_m.tile([128, SEQ], F32, tag="m", name="qproj_ps")
                for ec in range(EC):
                    nc.tensor.matmul(qp[:], wq_sb[:, h * EC + ec, :],
                                     inT(b, 1, ec),
                                     start=(ec == 0), stop=(ec == EC - 1))
                nc.vector.tensor_copy(qxT(b, h), qp[0:HID, :])

            def kproj_chain(b, sc, half):
                # flipped projection: stationary kT seq-chunk [128,128],
                # moving = 4-head weight block [128, 384]; lands kx natural.
                dst = kxo[b % 2][sc]
                hs = slice(half * 4 * HID, (half + 1) * 4 * HID)
                kp = ps_m.tile([128, 4, HID], F32, tag="m", name="kproj_ps")
                for ec in range(EC):
                    nc.tensor.matmul(
                        kp[:],
                        inT(b, 0, ec)[:, sc * 128:(sc + 1) * 128],
                        wk_sb[:, ec, hs],
                        start=(ec == 0), stop=(ec == EC - 1))
                nc.vector.tensor_copy(
                    dst[:, half * 4:(half + 1) * 4, 0:HID], kp[:])
                if half == 1:
                    nc.gpsimd.memset(dst[:, :, HID:HP], 1.0)

            def kxT_transpose_pair(b, hp):
                par = b % 2
                tp = ps_tr.tile([HID, 2, SEQ], BF16, tag="tr", name="kxT_tr")
                for h2 in range(2):
                    for sc in range(KC):
                        nc.tensor.transpose(
                            tp[:, h2, sc * 128:(sc + 1) * 128],
                            kxo[par][sc][:, hp * 2 + h2, 0:HID],
                            identity[:])
                nc.vector.tensor_copy(kxT_p[par][hp][:], tp[:])

            def scores_exp(b, h):
                # s^T (k on psum partitions) in kc-pair tiles; one exp()
                # per pair over 1024 columns.
                exp_sb = []
                for pair in range(2):
                    s2 = ps_s.tile([128, 2, SEQ], F32, tag="s",
                                   name="score_ps")
                    for k2 in range(2):
                        sc = pair * 2 + k2
                        nc.tensor.matmul(
                            s2[:, k2, :],
                            kxT(b, h)[:, sc * 128:(sc + 1) * 128],
                            qxT(b, h), start=True, stop=True)
                    e2 = exp_pool.tile([128, 2, SEQ], BF16, tag="exp",
                                       name="e_sb")
                    nc.scalar.activation(
                        e2[:], s2[:],
                        mybir.ActivationFunctionType.Exp, scale=SCALE)
                    exp_sb.extend([e2[:, 0, :], e2[:, 1, :]])
                return exp_sb

            def attn_head(b, h, exp_sb, store_final=False):
                # attention-weighted values + softmax denominator (col 96),
                # one [128, 4, 97] psum tile per head (all 4 q chunks);
                # normalisation: DVE evicts, gathers the 4 reciprocals in
                # one op, then scale-multiplies into the staging tile.
                # store_final: issue each qc's second-half store as soon as
                # its normalisation lands (trims the drain tail).
                par = b % 2
                o_ps = ps_m.tile([128, QC, HP], F32, tag="m", name="o_ps")
                for qc in range(QC):
                    for kc in range(KC):
                        nc.tensor.matmul(
                            o_ps[:, qc, :],
                            exp_sb[kc][:, qc * 128:(qc + 1) * 128],
                            kxo[par][kc][:, h, :],
                            start=(kc == 0), stop=(kc == KC - 1))
                if store_final:
                    # latency-optimised drain: per-qc recip/scale straight
                    # from PSUM (scale alternating DVE/ACT) + immediate
                    # store, so the final stores overlap the remaining
                    # normalisations
                    for qc in range(QC):
                        dst = stage[par][qc][:, h * HID:(h + 1) * HID]
                        rc = recip_pool.tile([128, 1], F32, tag="recip",
                                             name="recip")
                        nc.vector.reciprocal(rc[:], o_ps[:, qc, HID:HP])
                        if qc % 2:
                            nc.scalar.mul(dst, o_ps[:, qc, 0:HID], rc[:])
                        else:
                            nc.vector.tensor_scalar_mul(
                                dst, o_ps[:, qc, 0:HID], rc[:])
                        # alternate store queues (sync/scalar HWDGE) so the
                        # four ~600ns DMA issues don't serialize on one
                        # engine during the drain
                        eng = nc.scalar if qc % 2 else nc.sync
                        eng.dma_start(
                            out=out_d[b, qc * 128:(qc + 1) * 128,
                                      6 * HID:],
                            in_=stage[par][qc][:, 6 * HID:])
                    return
                o_sb = osb_pool.tile([128, QC, HP], F32, tag="osb",
                                     name="o_sb")
                nc.vector.tensor_copy(o_sb[:], o_ps[:])
                rc = recip_pool.tile([128, QC], F32, tag="recip",
                                     name="recip")
                nc.vector.reciprocal(rc[:], o_sb[:, :, HID:HP])
                for qc in range(QC):
                    nc.vector.tensor_scalar_mul(
                        stage[par][qc][:, h * HID:(h + 1) * HID],
                        o_sb[:, qc, 0:HID], rc[:, qc:qc + 1])

            def store_cols(b, lo, hi):
                par = b % 2
                sl = slice(lo, hi)
                for qc in range(QC):
                    nc.sync.dma_start(
                        out=out_d[b, qc * 128:(qc + 1) * 128, sl],
                        in_=stage[par][qc][:, sl])

            # --- emission -------------------------------------------------
            # batch 0 prologue (nothing earlier to hide behind); k-side
            # input transposes interleave the last q-projection heads so
            # their evictions overlap PE work.
            for ep in range(EC // 2):
                # alternate eviction engines so the three qT evictions
                # finish ~in parallel and the first q-projection can start
                input_transpose_pair(0, 1, nats[(0, 1)], ep,
                                     evict_on_scalar=(ep == 1))
            for h in range(N_HEAD):
                qproj_head(0, h)
                if h in (4, 5, 6):
                    # after the k cast has landed; evictions on the
                    # (startup-idle) scalar engine so the DVE's qxT
                    # eviction backlog can't delay kproj
                    input_transpose_pair(0, 0, nats[(0, 0)], h - 4,
                                         evict_on_scalar=True)

            for b in range(B):
                # PHASE K(b): k-projection chains interleaved with batch
                # b+1's input transposes (hides DVE eviction latency), then
                # per-head-pair kx^T transposes.
                # half-major order: all half-0 chains (heads 0-3) first, so
                # kx^T pairs 0/1 and the head-0 score prewarm can issue
                # right after — the scalar engine (idle during K) starts
                # exp() ~5us earlier, which is what gates the attention
                # phase (critical for the last batch, which has no
                # projection filler work).
                it = [(b + 1, t, nats[(b + 1, t)], ep)
                      for t in (1, 0) for ep in range(EC // 2)] \
                    if b + 1 < B else []
                ic = 0
                for half in range(2):
                    for sc in range(KC):
                        kproj_chain(b, sc, half)
                        if (half * KC + sc) % 3 and ic < len(it):
                            input_transpose_pair(*it[ic])
                            ic += 1
                    if half == 0:
                        kxT_transpose_pair(b, 0)
                        exps = {0: scores_exp(b, 0)}
                        kxT_transpose_pair(b, 1)
                while ic < len(it):
                    input_transpose_pair(*it[ic])
                    ic += 1
                for hp in range(2, N_HEAD // 2):
                    kxT_transpose_pair(b, hp)

                # PHASE A(b): qproj(b+1,h) -> scores(h+1) -> attn(h) per
                # slot; scores stay one head ahead of attn so exp latency
                # is covered by the interleaved projection work.
                for h in range(N_HEAD):
                    if b + 1 < B:
                        qproj_head(b + 1, h)
                    if h + 1 < N_HEAD:
                        exps[h + 1] = scores_exp(b, h + 1)
                    attn_head(b, h, exps.pop(h),
                              store_final=(h == N_HEAD - 1))
                    if h == 3:
                        store_cols(b, 0, EMBED // 2)
                    elif h == 5:
                        store_cols(b, EMBED // 2, 6 * HID)

    _split_excess_waits(nc)
    return nc


@lru_cache(maxsize=1)
def _get_nc():
    return build_bass()


def _pack_weights(w_kx, w_qx):
    # w_kx [8, 768, 96] -> [128, 6, 768]: (p, ec, h*96+d)
    wk = np.ascontiguousarray(
        w_kx.reshape(N_HEAD, EC, 128, HID).transpose(2, 1, 0, 3)
        .reshape(128, EC, N_HEAD * HID), dtype=np.float32)
    # w_qx [8, 768, 96] -> [128, 48, 128] zero-padded: (p, h*6+ec, d)
    wq = np.zeros((128, N_HEAD, EC, 128), dtype=np.float32)
    wq[:, :, :, 0:HID] = w_qx.reshape(N_HEAD, EC, 128, HID).transpose(
        2, 0, 1, 3)
    wq = np.ascontiguousarray(wq.reshape(128, N_HEAD * EC, 128))
    return wk, wq


def kernel(k, q, w_kx, w_qx):
    k = np.ascontiguousarray(k, dtype=np.float32)
    q = np.ascontiguousarray(q, dtype=np.float32)
    w_kx = np.ascontiguousarray(w_kx, dtype=np.float32)
    w_qx = np.ascontiguousarray(w_qx, dtype=np.float32)
    wk_packed, wq_packed = _pack_weights(w_kx, w_qx)

    nc = _get_nc()
    in_maps = []
    for c in range(N_CORES):
        sl = slice(c * B, (c + 1) * B)
        in_maps.append({
            "k": np.ascontiguousarray(k[sl]),
            "q": np.ascontiguousarray(q[sl]),
            "w_kx": wk_packed,
            "w_qx": wq_packed,
        })
    res = run_bass_kernel_spmd(nc, in_maps, core_ids=list(range(N_CORES)))
    return np.concatenate([res.results[c]["out"] for c in range(N_CORES)],
                          axis=0)


# revision 41
# speedup vs baseline: 1.0521x; 1.0356x over previous
"""Multi-head scaled-dot-product attention (ABSA-style, per-head projections)
on 8 Trainium2 NeuronCores.

Reference computation (per head h, batch b):
    kx = k @ w_kx[h]                    # (512, 96)
    qx = q @ w_qx[h]                    # (512, 96)
    s  = qx @ kx.T / sqrt(96)           # (512, 512)
    a  = softmax(s, axis=-1)
    o  = a @ kx                         # (512, 96)
    out[b, :, h*96:(h+1)*96] = o

Distribution: data-parallel over batch. 32 batches are split 4-per-core over
8 cores; every core holds the full (tiny) weights and computes all 8 heads
for its 4 batches. No collectives needed — the host concatenates the
per-core outputs.

Measured (8 axon-tunneled TRN2 NeuronCores, trace runs): ~178-184 us NEFF
exec (vs ~204-235 us for the v1 kernel under the same harness), L2 rel
err 3.9e-3.  ~84% TensorE occupancy; the PE instruction stream itself
(~157 us) is the wall — projections/scores/attention all issue within a
few ns of their streaming floor.

Key structure:
  - q-projection stationaries zero-padded to 128 cols host-side (FWL).
  - k-projection flipped: stationary = kT seq-chunk [128,128] (full PE
    width), moving = 4-head weight block [128,384] -> kx lands NATURAL
    (heads on columns), feeding the attention matmul directly; per-head
    PE transposes then produce kx^T for the scores.
  - Scores for kc pairs land in one [128,2,512] PSUM tile (2 banks) so a
    single ACTIVATE computes exp() over 1024 columns — halves the per-
    instruction overhead on the scalar engine (the attention-phase gate).
  - Softmax denominator via a ones-column folded into the attention
    matmul (col 96 of the kx tile, 97-stride heads).
  - Normalisation: DVE evicts attn PSUM to SBUF, strided-gathers the 4
    denominators into one reciprocal, then does the per-qc scale-
    multiplies SBUF->SBUF (fast DVE mode, no PSUM read penalty).
  - Software pipelining: batch b's attention phase interleaves batch
    b+1's q-projection (qproj(b+1,h) -> scores(h+1) -> attn(h) per
    slot, scores prewarmed one head at the end of phase K); batch
    b+1's input transposes interleave batch b's k-projection chains.
  - PSUM: scores 2x2 banks, shared pool (kproj/qproj/attn) 2 banks,
    transposes 2 banks = 8.
  - All PSUM evictions on the vector engine; scalar does exp() only.
"""

import math
from functools import lru_cache

import numpy as np

import concourse.bass as bass
import concourse.tile as tile
from concourse import mybir
from concourse.bass_utils import run_bass_kernel_spmd
from concourse.masks import make_identity

# ---------------------------------------------------------------------------
# Workaround for walrus "Too many sync wait commands": some instruction
# encodings accept only a single sync-wait, but Tile can attach several.
# Hoist every wait beyond the first onto a same-engine no-op inserted right
# before the instruction — program order on the engine makes that equivalent.
# ---------------------------------------------------------------------------

import bass_rust as _bass_rust


def _split_excess_waits(nc, max_waits=1):
    n = 0
    for f in nc.m.functions:
        for bb in f.blocks:
            il = bb.instructions
            i = 0
            while i < len(il):
                ins = il[i]
                si = ins.sync_info
                waits = list(si.on_wait or []) if si is not None else []
                if len(waits) > max_waits:
                    si.on_wait = waits[:max_waits]
                    for w in waits[max_waits:]:
                        nop = mybir.InstNoOp(name=f"waitnop-{n}", ins=[],
                                             outs=[])
                        n += 1
                        nop.engine = ins.engine
                        nop.sync_info = _bass_rust.SyncInfo(
                            on_wait=[w], on_update=[])
                        il.insert(i, nop)
                        i += 1
                i += 1

# ---------------------------------------------------------------------------
# Problem constants (full problem; hardcoded per the harness contract)
# ---------------------------------------------------------------------------
EMBED = 768
HID = 96
N_HEAD = 8
BATCH = 32
SEQ = 512
N_CORES = 8
B = BATCH // N_CORES  # batches per core
EC = EMBED // 128  # embed chunks of 128
KC = SEQ // 128  # key (seq) chunks of 128
QC = SEQ // 128  # query chunks of 128
SCALE = 1.0 / math.sqrt(HID)
HP = HID + 1  # per-head kxo stride: 96 data cols + 1 ones col

F32 = mybir.dt.float32
BF16 = mybir.dt.bfloat16


def build_bass():
    nc = bass.Bass("TRN2", target_bir_lowering=False, debug=False,
                   num_devices=N_CORES)

    k_in = nc.declare_dram_parameter("k", [B, SEQ, EMBED], F32, isOutput=False)
    q_in = nc.declare_dram_parameter("q", [B, SEQ, EMBED], F32, isOutput=False)
    # host-packed weights:
    #   w_kx: [128, EC, N_HEAD*HID]  (p, ec, h*96+d) = w_kx[h, ec*128+p, d]
    #   w_qx: [128, N_HEAD*EC, 128]  (p, h*6+ec, d)  = w_qx[h, ec*128+p, d],
    #         d-padded 96->128 with zeros (FWL wants 128 weight columns)
    # weights ship pre-cast to bf16 from the host: halves their DMA bytes,
    # which pulls the (serialised) k cast ~4us earlier at startup
    wk_in = nc.declare_dram_parameter("w_kx", [128, EC, N_HEAD * HID], BF16,
                                      isOutput=False)
    wq_in = nc.declare_dram_parameter("w_qx", [128, N_HEAD * EC, 128], BF16,
                                      isOutput=False)
    out_d = nc.declare_dram_parameter("out", [B, SEQ, EMBED], F32,
                                      isOutput=True)

    with nc.allow_low_precision("bf16 compute, f32 accumulate"), \
            tile.TileContext(nc) as tc:
        with tc.tile_pool(name="singles", bufs=1) as singles, \
                tc.tile_pool(name="nat", bufs=4) as nat_pool, \
                tc.tile_pool(name="kqt", bufs=1) as kqt_pool, \
                tc.tile_pool(name="wsb", bufs=1) as w_pool, \
                tc.tile_pool(name="stage", bufs=1) as stage_pool, \
                tc.tile_pool(name="exp", bufs=6) as exp_pool, \
                tc.tile_pool(name="osb", bufs=4) as osb_pool, \
                tc.tile_pool(name="recip", bufs=8) as recip_pool, \
                tc.tile_pool(name="ps_s", bufs=2, space="PSUM") as ps_s, \
                tc.tile_pool(name="ps_m", bufs=2, space="PSUM") as ps_m, \
                tc.tile_pool(name="ps_tr", bufs=2, space="PSUM") as ps_tr:

            # --- SBUF tiles -----------------------------------------------
            wq_sb = w_pool.tile([128, N_HEAD * EC, 128], BF16, tag="wq",
                                name="wq_sb")
            wk_sb = w_pool.tile([128, EC, N_HEAD * HID], BF16, tag="wk",
                                name="wk_sb")
            identity = singles.tile([128, 128], BF16, tag="identity")

            # qxT / kxT: head-PAIR tiles [96, 2, 512] bf16 (hid on parts).
            qxT_p = [[singles.tile([HID, 2, SEQ], BF16, tag=f"qxT_{i}_{hp}",
                                   name=f"qxT_{i}_{hp}")
                      for hp in range(N_HEAD // 2)] for i in range(2)]
            # kxo: per (parity, seq-chunk) [128, N_HEAD, 97] bf16 — kx in
            # natural layout, ones column at 96 (softmax denominator).
            kxo = [[singles.tile([128, N_HEAD, HP], BF16,
                                 tag=f"kxo_{i}_{sc}", name=f"kxo_{i}_{sc}")
                    for sc in range(KC)] for i in range(2)]
            kxT_p = [[singles.tile([HID, 2, SEQ], BF16, tag=f"kxT_{i}_{hp}",
                                   name=f"kxT_{i}_{hp}")
                      for hp in range(N_HEAD // 2)] for i in range(2)]
            stage = [[stage_pool.tile([128, EMBED], F32, tag=f"st{p}_{qc}",
                                      name=f"st{p}_{qc}")
                      for qc in range(QC)] for p in range(2)]

            def qxT(b, h):
                return qxT_p[b % 2][h // 2][:, h % 2, :]

            def kxT(b, h):
                return kxT_p[b % 2][h // 2][:, h % 2, :]

            # --- input pipeline -------------------------------------------
            # SWDGE cast-DMAs (f32 -> bf16, contiguous descriptors).  Queue
            # order: wq half 0 first (smallest gating load), then q0, then
            # identity prep, k0, wq half 1, wk, batches 1..3.
            def load_wq(half):
                hb = N_HEAD * EC // 2
                sl = slice(half * hb, (half + 1) * hb)
                nc.gpsimd.dma_start(out=wq_sb[:, sl, :], in_=wq_in[:, sl, :])

            def load_wk():
                nc.gpsimd.dma_start(out=wk_sb[:], in_=wk_in[:])

            def cast_batch_tensor(b, t, split=False):
                src_d = (k_in, q_in)[t]
                nat = nat_pool.tile([128, KC, EMBED], BF16,
                                    tag=f"nat{t}", name=f"nat{t}_{b}")
                src = src_d[b].rearrange("(kc p) e -> p kc e", p=128)
                if split:
                    # two cast-DMAs so the first seq-half lands ~2us
                    # earlier and its transposes can start (Tile's
                    # region-level deps release them per kc chunk)
                    half = KC // 2
                    nc.gpsimd.dma_start(out=nat[:, 0:half, :],
                                        in_=src[:, 0:half, :])
                    nc.gpsimd.dma_start(out=nat[:, half:, :],
                                        in_=src[:, half:, :])
                else:
                    nc.gpsimd.dma_start(out=nat[:], in_=src[:])
                return nat

            load_wq(0)
            nat_q0 = cast_batch_tensor(0, 1)
            make_identity(nc, identity[:])
            load_wq(1)
            nat_k0 = cast_batch_tensor(0, 0)
            load_wk()
            nats = {(0, 1): nat_q0, (0, 0): nat_k0}
            for b in range(1, B):
                for t in (1, 0):
                    nats[(b, t)] = cast_batch_tensor(b, t)

            # PE warm-up transposes: keep the PE busy from engine-ready to
            # first real matmul so the HAM clock gate flips to 2.4 GHz and
            # stays there.
            warm_ps = ps_s.tile([128, 256], BF16, tag="s", name="warm_ps")
            for _ in range(88):
                nc.tensor.transpose(warm_ps[:, 0:128], identity[:],
                                    identity[:])

            # kT/qT (embed on partitions) built with PE transposes, stored
            # as ec-PAIR tiles [128, 2, 512] bf16.
            kqt = {}

            def inT(b, t, ec):
                return kqt[(b, t, ec // 2)][:, ec % 2, :]

            def input_transpose_pair(b, t, nat, ep, evict_on_scalar=False):
                tp = ps_tr.tile([128, 2, KC, 128], BF16, tag="tr",
                                name="in_tr")
                for e2 in range(2):
                    for kc in range(KC):
                        nc.tensor.transpose(
                            tp[:, e2, kc, :],
                            nat[:, kc, (ep * 2 + e2) * 128:
                                (ep * 2 + e2 + 1) * 128],
                            identity[:])
                tt = kqt_pool.tile([128, 2, SEQ], BF16,
                                   tag=f"T{t}_{b}_{ep}",
                                   name=f"T{t}_{b}_{ep}")
                if evict_on_scalar:
                    nc.scalar.copy(tt[:], tp[:])
                else:
                    nc.vector.tensor_copy(tt[:], tp[:])
                kqt[(b, t, ep)] = tt

            # --- phase building blocks ------------------------------------
            def qproj_head(b, h):
                # qx^T[h] via padded stationary wq chunk [128,128]:
                # psum rows 0:96 = qx^T, rows 96:128 = zeros (pad).
                qp = ps_m.tile([128, SEQ], F32, tag="m", name="qproj_ps")
                for ec in range(EC):
                    nc.tensor.matmul(qp[:], wq_sb[:, h * EC + ec, :],
                                     inT(b, 1, ec),
                                     start=(ec == 0), stop=(ec == EC - 1))
                nc.vector.tensor_copy(qxT(b, h), qp[0:HID, :])

            def kproj_chain(b, sc, half):
                # flipped projection: stationary kT seq-chunk [128,128],
                # moving = 4-head weight block [128, 384]; lands kx natural.
                dst = kxo[b % 2][sc]
                hs = slice(half * 4 * HID, (half + 1) * 4 * HID)
                kp = ps_m.tile([128, 4, HID], F32, tag="m", name="kproj_ps")
                for ec in range(EC):
                    nc.tensor.matmul(
                        kp[:],
                        inT(b, 0, ec)[:, sc * 128:(sc + 1) * 128],
                        wk_sb[:, ec, hs],
                        start=(ec == 0), stop=(ec == EC - 1))
                nc.vector.tensor_copy(
                    dst[:, half * 4:(half + 1) * 4, 0:HID], kp[:])
                if half == 1:
                    nc.gpsimd.memset(dst[:, :, HID:HP], 1.0)

            def kxT_transpose_pair(b, hp):
                par = b % 2
                tp = ps_tr.tile([HID, 2, SEQ], BF16, tag="tr", name="kxT_tr")
                for h2 in range(2):
                    for sc in range(KC):
                        nc.tensor.transpose(
                            tp[:, h2, sc * 128:(sc + 1) * 128],
                            kxo[par][sc][:, hp * 2 + h2, 0:HID],
                            identity[:])
                nc.vector.tensor_copy(kxT_p[par][hp][:], tp[:])

            def scores_exp(b, h):
                # s^T (k on psum partitions) in kc-pair tiles; one exp()
                # per pair over 1024 columns.
                exp_sb = []
                for pair in range(2):
                    s2 = ps_s.tile([128, 2, SEQ], F32, tag="s",
                                   name="score_ps")
                    for k2 in range(2):
                        sc = pair * 2 + k2
                        nc.tensor.matmul(
                            s2[:, k2, :],
                            kxT(b, h)[:, sc * 128:(sc + 1) * 128],
                            qxT(b, h), start=True, stop=True)
                    e2 = exp_pool.tile([128, 2, SEQ], BF16, tag="exp",
                                       name="e_sb")
                    nc.scalar.activation(
                        e2[:], s2[:],
                        mybir.ActivationFunctionType.Exp, scale=SCALE)
                    exp_sb.extend([e2[:, 0, :], e2[:, 1, :]])
                return exp_sb

            def attn_head(b, h, exp_sb, store_final=False):
                # attention-weighted values + softmax denominator (col 96),
                # one [128, 4, 97] psum tile per head (all 4 q chunks);
                # normalisation: DVE evicts, gathers the 4 reciprocals in
                # one op, then scale-multiplies into the staging tile.
                # store_final: issue each qc's second-half store as soon as
                # its normalisation lands (trims the drain tail).
                par = b % 2
                o_ps = ps_m.tile([128, QC, HP], F32, tag="m", name="o_ps")
                for qc in range(QC):
                    for kc in range(KC):
                        nc.tensor.matmul(
                            o_ps[:, qc, :],
                            exp_sb[kc][:, qc * 128:(qc + 1) * 128],
                            kxo[par][kc][:, h, :],
                            start=(kc == 0), stop=(kc == KC - 1))
                if store_final:
                    # latency-optimised drain: per-qc recip/scale straight
                    # from PSUM (scale alternating DVE/ACT) + immediate
                    # store, so the final stores overlap the remaining
                    # normalisations
                    for qc in range(QC):
                        dst = stage[par][qc][:, h * HID:(h + 1) * HID]
                        rc = recip_pool.tile([128, 1], F32, tag="recip",
                                             name="recip")
                        nc.vector.reciprocal(rc[:], o_ps[:, qc, HID:HP])
                        if qc % 2:
                            nc.scalar.mul(dst, o_ps[:, qc, 0:HID], rc[:])
                        else:
                            nc.vector.tensor_scalar_mul(
                                dst, o_ps[:, qc, 0:HID], rc[:])
                        # alternate store queues (sync/scalar HWDGE) so the
                        # four ~600ns DMA issues don't serialize on one
                        # engine during the drain
                        eng = nc.scalar if qc % 2 else nc.sync
                        eng.dma_start(
                            out=out_d[b, qc * 128:(qc + 1) * 128,
                                      6 * HID:],
                            in_=stage[par][qc][:, 6 * HID:])
                    return
                o_sb = osb_pool.tile([128, QC, HP], F32, tag="osb",
                                     name="o_sb")
                nc.vector.tensor_copy(o_sb[:], o_ps[:])
                rc = recip_pool.tile([128, QC], F32, tag="recip",
                                     name="recip")
                nc.vector.reciprocal(rc[:], o_sb[:, :, HID:HP])
                for qc in range(QC):
                    nc.vector.tensor_scalar_mul(
                        stage[par][qc][:, h * HID:(h + 1) * HID],
                        o_sb[:, qc, 0:HID], rc[:, qc:qc + 1])

            def store_cols(b, lo, hi):
                par = b % 2
                sl = slice(lo, hi)
                for qc in range(QC):
                    nc.sync.dma_start(
                        out=out_d[b, qc * 128:(qc + 1) * 128, sl],
                        in_=stage[par][qc][:, sl])

            # --- emission -------------------------------------------------
            # batch 0 prologue (nothing earlier to hide behind); k-side
            # input transposes interleave the last q-projection heads so
            # their evictions overlap PE work.
            for ep in range(EC // 2):
                # alternate eviction engines so the three qT evictions
                # finish ~in parallel and the first q-projection can start
                input_transpose_pair(0, 1, nats[(0, 1)], ep,
                                     evict_on_scalar=(ep == 1))
            for h in range(N_HEAD):
                qproj_head(0, h)
                if h in (4, 5, 6):
                    # after the k cast has landed; evictions on the
                    # (startup-idle) scalar engine so the DVE's qxT
                    # eviction backlog can't delay kproj
                    input_transpose_pair(0, 0, nats[(0, 0)], h - 4,
                                         evict_on_scalar=True)

            for b in range(B):
                # PHASE K(b): k-projection chains interleaved with batch
                # b+1's input transposes (hides DVE eviction latency), then
                # per-head-pair kx^T transposes.
                # half-major order: all half-0 chains (heads 0-3) first, so
                # kx^T pairs 0/1 and the head-0 score prewarm can issue
                # right after — the scalar engine (idle during K) starts
                # exp() ~5us earlier, which is what gates the attention
                # phase (critical for the last batch, which has no
                # projection filler work).
                it = [(b + 1, t, nats[(b + 1, t)], ep)
                      for t in (1, 0) for ep in range(EC // 2)] \
                    if b + 1 < B else []
                ic = 0
                for half in range(2):
                    for sc in range(KC):
                        kproj_chain(b, sc, half)
                        if (half * KC + sc) % 3 and ic < len(it):
                            input_transpose_pair(*it[ic])
                            ic += 1
                    if half == 0:
                        kxT_transpose_pair(b, 0)
                        exps = {0: scores_exp(b, 0)}
                        kxT_transpose_pair(b, 1)
                while ic < len(it):
                    input_transpose_pair(*it[ic])
                    ic += 1
                for hp in range(2, N_HEAD // 2):
                    kxT_transpose_pair(b, hp)

                # PHASE A(b): qproj(b+1,h) -> scores(h+1) -> attn(h) per
                # slot; scores stay one head ahead of attn so exp latency
                # is covered by the interleaved projection work.
                for h in range(N_HEAD):
                    if b + 1 < B:
                        qproj_head(b + 1, h)
                    if h + 1 < N_HEAD:
                        exps[h + 1] = scores_exp(b, h + 1)
                    attn_head(b, h, exps.pop(h),
                              store_final=(h == N_HEAD - 1))
                    if h == 3:
                        store_cols(b, 0, EMBED // 2)
                    elif h == 5:
                        store_cols(b, EMBED // 2, 6 * HID)

    _split_excess_waits(nc)
    return nc


@lru_cache(maxsize=1)
def _get_nc():
    return build_bass()


def _pack_weights(w_kx, w_qx):
    import ml_dtypes
    bf16 = ml_dtypes.bfloat16
    # w_kx [8, 768, 96] -> [128, 6, 768] bf16: (p, ec, h*96+d)
    wk = np.ascontiguousarray(
        w_kx.reshape(N_HEAD, EC, 128, HID).transpose(2, 1, 0, 3)
        .reshape(128, EC, N_HEAD * HID).astype(bf16))
    # w_qx [8, 768, 96] -> [128, 48, 128] bf16 zero-padded: (p, h*6+ec, d)
    wq = np.zeros((128, N_HEAD, EC, 128), dtype=bf16)
    wq[:, :, :, 0:HID] = w_qx.reshape(N_HEAD, EC, 128, HID).transpose(
        2, 0, 1, 3).astype(bf16)
    wq = np.ascontiguousarray(wq.reshape(128, N_HEAD * EC, 128))
    return wk, wq


def kernel(k, q, w_kx, w_qx):
    k = np.ascontiguousarray(k, dtype=np.float32)
    q = np.ascontiguousarray(q, dtype=np.float32)
    w_kx = np.ascontiguousarray(w_kx, dtype=np.float32)
    w_qx = np.ascontiguousarray(w_qx, dtype=np.float32)
    wk_packed, wq_packed = _pack_weights(w_kx, w_qx)

    nc = _get_nc()
    in_maps = []
    for c in range(N_CORES):
        sl = slice(c * B, (c + 1) * B)
        in_maps.append({
            "k": np.ascontiguousarray(k[sl]),
            "q": np.ascontiguousarray(q[sl]),
            "w_kx": wk_packed,
            "w_qx": wq_packed,
        })
    res = run_bass_kernel_spmd(nc, in_maps, core_ids=list(range(N_CORES)))
    return np.concatenate([res.results[c]["out"] for c in range(N_CORES)],
                          axis=0)
